# revision 15
# baseline (speedup 1.0000x reference)
"""Trainium2 Bass kernel for CFConv-style GNN message passing layer.

Full computation (see reference):
  smeared = exp(coeff*(d - offset)^2)            [E, 50]
  W = (relu(smeared @ enn1_w.T + b1) @ enn2_w.T + b2) * C(d)   [E, 64]
  h = x @ lin1_w.T                                [N, 64]
  agg = segment_sum(h[src] * W, dst, N)           [N, 64]
  out = x + relu(agg @ lin2_w.T + lin2_b)         [N, 128]

Sharding: edges partitioned by dst range across 8 cores. Each core computes
h for its node slice, AllGathers h, then processes its own edges.
Edge stream per core is sorted by (src bucket, dst block of 128, src):
  - gather of h rows via gpsimd dma_gather (int16 idx per 32k-row bucket)
  - scatter-add via one-hot matmul: for each 128-edge chunk,
    D[e, m] = (dst_local[e] == m + 128*k) built on DVE via is_equal against
    iota constants; agg_block [128,64] += D.T @ msg accumulated in PSUM
    across the chunks of one (bucket, block) cell. Cells straddling chunk
    boundaries use offset index k>0 for the partial head chunk.
Cell capacities are the exact max across cores so the instruction stream is
SPMD-uniform. The smearing quadratic is evaluated with a K=9 bf16 matmul
using split-precision rows (exact to ~1e-4 in the exponent).
"""

import math

import numpy as np

N_NODES = 100000
N_EDGES = 1600000
DIM = 128
NF = 64
NG = 50
CUTOFF = 5.0
N_CORES = 8

GRP = 4096                      # slots per gather group (= 2 pair-tiles)
CPG = GRP // 128                # chunks per group


def _cfg(n_nodes, n_cores):
    npc = n_nodes // n_cores            # real nodes per core
    assert npc * n_cores == n_nodes
    npad = ((npc + 127) // 128) * 128    # padded nodes per core
    nbg = npad * n_cores                 # global padded node count
    n_buckets = 4
    assert nbg % n_buckets == 0
    bkt = nbg // n_buckets               # gather bucket size (int16 range)
    assert bkt <= 32767
    nsb = npad // 128                    # dst blocks of 128 per core
    return dict(npc=npc, npad=npad, nbg=nbg, n_buckets=n_buckets, bkt=bkt,
                nsb=nsb)


def _bf(a):
    import ml_dtypes
    return a.astype(ml_dtypes.bfloat16)


def _split3(a):
    """Split fp64 array into three bf16 terms summing to ~2^-27 rel."""
    h = _bf(a)
    r = a - h.astype(np.float64)
    m = _bf(r)
    l = _bf(r - m.astype(np.float64))
    return h, m, l


def prep_host(x, edge_index, edge_weight, edge_attr, n_cores=N_CORES):
    """Shard + reorder edges; build per-core input arrays + chunk metadata."""
    cfg = _cfg(x.shape[0], n_cores)
    npc, npad, n_buckets, bkt, nsb = (cfg["npc"], cfg["npad"],
                                      cfg["n_buckets"], cfg["bkt"], cfg["nsb"])

    src = np.asarray(edge_index[0], dtype=np.int64)
    dst = np.asarray(edge_index[1], dtype=np.int64)
    d = np.asarray(edge_attr, dtype=np.float32)

    # global padded src ids laid out to match the two-stage AllGather:
    # stage 0 gathers quarter-slice 0 of every core into rows [0, bkt);
    # stage 1 gathers the remaining 3 quarters into rows [bkt, nbg).
    qn = npad // 4
    bktg = npad * n_cores // 4
    lp = src % npc
    cr = src // npc
    gsrc = np.where(lp < qn, cr * qn + lp,
                    np.where(lp < 2 * qn, bktg + cr * qn + (lp - qn),
                             2 * bktg + cr * 2 * qn + (lp - 2 * qn)))
    core_of = dst // npc

    per_core = []
    counts = np.zeros((n_cores, n_buckets, nsb), dtype=np.int64)
    for r in range(n_cores):
        m = core_of == r
        gs = gsrc[m]
        dl = (dst[m] - r * npc).astype(np.int64)
        dd = d[m]
        bu = gs // bkt
        sb = dl // 128
        order = np.lexsort((gs, sb, bu))
        gs, dl, dd, bu, sb = gs[order], dl[order], dd[order], bu[order], sb[order]
        np.add.at(counts[r], (bu, sb), 1)
        per_core.append((gs, dl, dd, bu, sb))

    caps = counts.max(axis=0)                              # [n_buckets, nsb]
    bucket_len = caps.sum(axis=1)
    bucket_pad = ((bucket_len + GRP - 1) // GRP) * GRP
    bucket_base = np.concatenate([[0], np.cumsum(bucket_pad)[:-1]])
    cell_start = np.zeros((n_buckets, nsb), dtype=np.int64)
    for b in range(n_buckets):
        cell_start[b] = bucket_base[b] + np.concatenate(
            [[0], np.cumsum(caps[b])[:-1]])
    e_pad = int(bucket_pad.sum())
    nchunks = e_pad // 128
    ngroup = e_pad // GRP

    # segment metadata (identical across cores): per chunk, the list of
    # cell segments [sb, k, first, last]
    chunk_segs = [[] for _ in range(nchunks)]
    for b in range(n_buckets):
        for s in range(nsb):
            cap = int(caps[b, s])
            if cap == 0:
                continue
            p0 = int(cell_start[b, s])
            p1 = p0 + cap
            c0, c1 = p0 // 128, (p1 - 1) // 128
            for c in range(c0, c1 + 1):
                chunk_segs[c].append([s, 0, c == c0, c == c1, p0, p1])
    slot_off = np.zeros(e_pad, dtype=np.float32)
    max_k = 0
    for c, segs in enumerate(chunk_segs):
        segs.sort(key=lambda t: max(t[4], c * 128))
        for i, t in enumerate(segs):
            k = 0 if t[4] <= c * 128 else i
            t[1] = k
            max_k = max(max_k, k)
            if k:
                a = max(t[4], c * 128)
                z = min(t[5], (c + 1) * 128)
                slot_off[a:z] = 128.0 * k
    assert max_k <= 2, max_k
    # force cell splits at gather-group boundaries
    for g in range(1, ngroup):
        c = g * CPG
        for t in chunk_segs[c]:
            if not t[2] and t[4] < c * 128:
                t[2] = True
                for t2 in chunk_segs[c - 1]:
                    if t2[4] == t[4]:
                        t2[3] = True

    cfg["e_pad"] = e_pad
    cfg["ngroup"] = ngroup
    cfg["chunk_segs"] = chunk_segs
    cfg["group_bucket"] = np.searchsorted(
        np.cumsum(bucket_pad), np.arange(ngroup) * GRP, side="right")
    # group after which each y-iteration (pair of dst blocks) is ready
    last_group = np.zeros(nsb, dtype=np.int64)
    for s in range(nsb):
        for b in range(n_buckets):
            if caps[b, s] > 0:
                last_group[s] = max(
                    last_group[s],
                    (int(cell_start[b, s]) + int(caps[b, s]) - 1) // GRP)
    y_of_group = [[] for _ in range(ngroup)]
    for t in range(nsb // 2):
        y_of_group[max(last_group[2 * t], last_group[2 * t + 1])].append(t)
    cfg["y_of_group"] = y_of_group

    nt = e_pad // 2048  # pair-tiles
    cell_flat = cell_start.reshape(-1)
    coeff = -0.5 / (CUTOFF / (NG - 1)) ** 2
    ins = []
    for r in range(n_cores):
        gs, dl, dd, bu, sb = per_core[r]
        key = bu * nsb + sb
        n = len(key)
        if n:
            newgrp = np.r_[True, key[1:] != key[:-1]]
            starts = np.nonzero(newgrp)[0]
            lens = np.diff(np.r_[starts, n])
            cc = np.arange(n) - np.repeat(starts, lens)
        else:
            cc = np.zeros(0, dtype=np.int64)
        pos = cell_flat[key] + cc

        srcl = np.zeros(e_pad, dtype=np.int16)
        dloc = np.full(e_pad, 99999.0, dtype=np.float32)
        dpad = np.full(e_pad, CUTOFF, dtype=np.float32)
        srcl[pos] = (gs - bu * bkt).astype(np.int16)
        dloc[pos] = (dl % 128).astype(np.float32)
        dloc += slot_off
        dpad[pos] = dd

        d64 = dpad.astype(np.float64)
        cw = (0.5 * (np.cos(dpad * (math.pi / CUTOFF)) + 1.0)).astype(np.float32)
        uh, um, ul = _split3(coeff * d64 * d64)
        vh, vm, vl = _split3(-2.0 * coeff * d64)
        # poly rows per half: [uh um ul vh vm vl vh vm vh]  (9 bf16 rows)
        rows = [uh, um, ul, vh, vm, vl, vh, vm, vh]
        poly = np.empty((nt, 18, 1024), dtype=uh.dtype)
        for i, rr_ in enumerate(rows):
            rv = rr_.reshape(nt, 2, 1024)
            poly[:, i, :] = rv[:, 0, :]
            poly[:, 9 + i, :] = rv[:, 1, :]
        xt = np.zeros((DIM, npad), dtype=np.float32)
        xt[:, :npc] = np.asarray(x[r * npc:(r + 1) * npc, :], np.float32).T
        ins.append(dict(
            xT=np.ascontiguousarray(xt),
            poly=np.ascontiguousarray(poly),
            cw=cw,
            srcidx=np.ascontiguousarray(np.tile(srcl.reshape(-1, 16).T, (8, 1))),
            dstloc=np.ascontiguousarray(dloc.reshape(-1, 128).T),
        ))
    return cfg, ins


def _stack2(w, rows):
    out = np.zeros((rows, w.shape[1]), dtype=np.float32)
    out[:w.shape[0]] = w
    out[64:64 + w.shape[0]] = w
    return out


def prep_weights(lin1_w, lin2_w, lin2_b, enn1_w, enn1_b, enn2_w, enn2_b):
    """Constant (per-core-identical) weight arrays."""
    offset = np.linspace(0.0, CUTOFF, NG).astype(np.float64)
    coeff = -0.5 / (offset[1] - offset[0]) ** 2
    oh, om, ol = _split3(offset)
    one = np.ones(NG, dtype=np.float64)
    # lhsT rows pair with poly rows: [1 1 1 oh oh oh om om ol]
    lrows = [one, one, one, oh, oh, oh, om, om, ol]
    poly_lhsT = np.zeros((73, 64), dtype=np.float32)
    for b0 in (0, 64):
        for i, lr in enumerate(lrows):
            poly_lhsT[b0 + i, :NG] = np.asarray(lr, np.float32)
    eb = np.full((128, 1), -88.0, dtype=np.float32)
    eb[:NG, 0] = (coeff * offset * offset).astype(np.float32)
    eb[64:64 + NG, 0] = eb[:NG, 0]
    b1s = np.zeros((128, 1), dtype=np.float32)
    b1s[:NF, 0] = enn1_b
    b1s[64:64 + NF, 0] = enn1_b
    b2s = np.zeros((128, 1), dtype=np.float32)
    b2s[:NF, 0] = enn2_b
    b2s[64:64 + NF, 0] = enn2_b
    iota8 = np.tile(np.arange(128, dtype=np.float32)[None, :], (128, 8))
    iotah = np.tile(np.arange(128, 384, dtype=np.float32)[None, :], (128, 1))
    return dict(
        lin1_wT=np.ascontiguousarray(lin1_w.T.astype(np.float32)),    # [128, 64]
        lin2_wT=np.ascontiguousarray(lin2_w.T.astype(np.float32)),    # [64, 128]
        enn1_wT=_stack2(enn1_w.T.astype(np.float32), 114),            # [114, 64]
        enn2_wT=_stack2(enn2_w.T.astype(np.float32), 128),            # [128, 64]
        poly_lhsT=poly_lhsT,
        eb=eb, b1s=b1s, b2s=b2s,
        ident=np.eye(128, dtype=np.float32),
        iota8=np.ascontiguousarray(iota8),                            # [128, 1024]
        iotah=np.ascontiguousarray(iotah),                            # [128, 256]
        l2b=np.ascontiguousarray(lin2_b.astype(np.float32).reshape(128, 1)),
    )


def build_nc(cfg, n_cores=N_CORES, **_ignored):
    import concourse.bass as bass
    import concourse.bacc as bacc
    import concourse.mybir as mybir
    import concourse.tile as tile
    from concourse import library_config

    f32 = mybir.dt.float32
    bf16 = mybir.dt.bfloat16
    i16 = mybir.dt.int16
    npad, nbg, bkt, e_pad = cfg["npad"], cfg["nbg"], cfg["bkt"], cfg["e_pad"]
    nsb = cfg["nsb"]
    NT = e_pad // 2048            # pair-tiles
    NGROUP = cfg["ngroup"]
    chunk_segs = cfg["chunk_segs"]
    group_bucket = cfg["group_bucket"]

    nc = bacc.Bacc(None, num_devices=n_cores)

    # I/O
    xT_d = nc.dram_tensor("xT", [DIM, npad], f32, kind="ExternalInput")
    poly_d = nc.dram_tensor("poly", [NT, 18, 1024], bf16, kind="ExternalInput")
    cw_d = nc.dram_tensor("cw", [e_pad], f32, kind="ExternalInput")
    sidx_d = nc.dram_tensor("srcidx", [128, e_pad // 16], i16, kind="ExternalInput")
    dstloc_d = nc.dram_tensor("dstloc", [128, e_pad // 128], f32,
                              kind="ExternalInput")
    w_d = {}
    for name, shape in [("lin1_wT", [DIM, NF]), ("lin2_wT", [NF, DIM]),
                        ("enn1_wT", [114, NF]), ("enn2_wT", [128, NF]),
                        ("poly_lhsT", [73, 64]), ("eb", [128, 1]),
                        ("b1s", [128, 1]), ("b2s", [128, 1]), ("l2b", [128, 1]),
                        ("ident", [128, 128]), ("iota8", [128, 1024]),
                        ("iotah", [128, 256])]:
        w_d[name] = nc.dram_tensor(name, shape, f32, kind="ExternalInput")
    out_d = nc.dram_tensor("out", [DIM, npad], f32, kind="ExternalOutput")

    h_self = nc.dram_tensor("h_self", [npad, NF], f32)
    h_full = nc.dram_tensor("h_full", [nbg, NF], f32)

    with tile.TileContext(nc) as tc:
        with tc.tile_pool(name="const", bufs=1) as cp:
            wt = {}
            for name in w_d:
                t = cp.tile(list(w_d[name].shape), f32, tag='w_' + name)
                nc.sync.dma_start(out=t[:], in_=w_d[name][:, :])
                wt[name] = t
            ident = wt["ident"]
            # bf16 copies for the bf16 matmul path
            wb = {}
            for name in ("enn1_wT", "enn2_wT", "ident", "poly_lhsT"):
                t = cp.tile(list(w_d[name].shape), bf16, tag='wb_' + name)
                nc.scalar.activation(t[:], wt[name][:],
                                     mybir.ActivationFunctionType.Copy)
                wb[name] = t
            ident_bf = wb["ident"]
            nc.gpsimd.load_library(library_config.mlp)
            agg = cp.tile([128, nsb, NF], f32, tag='agg')
            nc.vector.memset(agg[:], 0.0)

            # ---------------- h phase ----------------
            with (tc.tile_pool(name="hp", bufs=3) as hp,
                  tc.tile_pool(name="hpp", bufs=2, space="PSUM") as hpp):
                for s in range(npad // 256):
                    xt = hp.tile([128, 256], f32, tag='xt')
                    nc.sync.dma_start(out=xt[:], in_=xT_d[:, s * 256:(s + 1) * 256])
                    hps = hpp.tile([128, 128], f32, tag='hps')
                    for c in range(2):
                        nc.tensor.matmul(
                            hps[:, c * 64:(c + 1) * 64],
                            lhsT=xt[:, c * 128:(c + 1) * 128],
                            rhs=wt["lin1_wT"][:],
                            start=True, stop=True)
                    hsb = hp.tile([128, 128], f32, tag='hsb')
                    nc.scalar.activation(hsb[:], hps[:],
                                         mybir.ActivationFunctionType.Copy)
                    dst_ap = h_self[s * 256:(s + 1) * 256, :].rearrange(
                        "(c p) f -> p c f", p=128)
                    nc.sync.dma_start(
                        out=dst_ap,
                        in_=hsb[:].rearrange("p (c f) -> p c f", f=NF))

            qn = npad // 4
            for (a, z, oa, oz) in ((0, qn, 0, bkt), (qn, 2 * qn, bkt, 2 * bkt),
                                   (2 * qn, npad, 2 * bkt, nbg)):
                nc.gpsimd.collective_compute(
                    "AllGather", mybir.AluOpType.bypass,
                    replica_groups=[list(range(n_cores))],
                    ins=[h_self[a:z, :].opt()],
                    outs=[h_full[oa:oz, :].opt()])

            # ---------------- edge phase ----------------
            with (tc.tile_pool(name="gp", bufs=4) as gp,
                  tc.tile_pool(name="ep", bufs=2) as ep,
                  tc.tile_pool(name="yp", bufs=2) as yp,
                  tc.tile_pool(name="pp_a", bufs=1, space="PSUM") as pp_a,
                  tc.tile_pool(name="pp_h1", bufs=1, space="PSUM") as pp_h1,
                  tc.tile_pool(name="pp_wc", bufs=1, space="PSUM") as pp_wc,
                  tc.tile_pool(name="pp_agg", bufs=1, space="PSUM") as pp_agg):
                pend = {}
                cur_psum = [None]

                def y_iter(s):
                    yt = pp_agg.tile([128, 512], f32, name='yt', tag='ytile')
                    atp = yt[0:64, 256:512]
                    for j in range(2):
                        blk = 2 * s + j
                        nc.tensor.transpose(
                            atp[:, j * 128:(j + 1) * 128], agg[:, blk, :],
                            ident[:])
                    ats = yp.tile([64, 256], f32, name='ats', tag='ats')
                    nc.scalar.activation(ats[:], atp[:],
                                         mybir.ActivationFunctionType.Copy)
                    ytp = yt[:, 0:256]
                    nc.tensor.matmul(ytp, lhsT=wt["lin2_wT"][:],
                                     rhs=ats[:], start=True, stop=True)
                    yr = yp.tile([128, 256], f32, name='yr', tag='yr')
                    nc.scalar.activation(yr[:], ytp,
                                         mybir.ActivationFunctionType.Relu,
                                         bias=wt["l2b"][:])
                    xt2 = yp.tile([128, 256], f32, name='xt2', tag='xt2')
                    nc.sync.dma_start(out=xt2[:],
                                      in_=xT_d[:, s * 256:(s + 1) * 256])
                    ot = yp.tile([128, 256], f32, name='ot', tag='ot')
                    nc.vector.tensor_tensor(out=ot[:], in0=yr[:], in1=xt2[:],
                                            op=mybir.AluOpType.add)
                    nc.sync.dma_start(out=out_d[:, s * 256:(s + 1) * 256],
                                      in_=ot[:])

                def load_group(g):
                    b = int(group_bucket[g])
                    sidx = gp.tile([128, 256], i16, tag='sidx')
                    nc.sync.dma_start(
                        out=sidx[:],
                        in_=sidx_d[:, g * 256:(g + 1) * 256])
                    dstl = gp.tile([128, 32], f32, tag='dstl')
                    nc.sync.dma_start(
                        out=dstl[:],
                        in_=dstloc_d[:, g * 32:(g + 1) * 32])
                    cwt = gp.tile([128, 32], f32, tag='cwt')
                    nc.sync.dma_start(
                        out=cwt[:],
                        in_=cw_d[g * 4096:(g + 1) * 4096].rearrange(
                            "(c p) -> p c", p=128))
                    gath = gp.tile([128, 32, NF], f32, tag='gath')
                    nc.gpsimd.dma_gather(
                        gath[:], h_full[b * bkt:(b + 1) * bkt, :], sidx[:],
                        num_idxs=4096, num_idxs_reg=4096, elem_size=NF,
                        single_packet=False)
                    pend[g] = (gath, dstl, cwt)

                def compute_group(g):
                    gath, dstl, cwt = pend.pop(g)
                    msg = ep.tile([128, 32, NF], bf16, tag='msg')
                    dmat = ep.tile([128, 32, 128], bf16, tag='dmat')
                    iota_v = wt["iota8"][:].rearrange("p (c k) -> p c k", k=128)
                    iotah_v = wt["iotah"][:].rearrange("p (c k) -> p c k", k=128)
                    for half in range(2):
                        t = 2 * g + half
                        poly = ep.tile([73, 1024], bf16, tag='poly')
                        nc.sync.dma_start(out=poly[0:9, :], in_=poly_d[t, 0:9, :])
                        nc.sync.dma_start(out=poly[64:73, :],
                                          in_=poly_d[t, 9:18, :])
                        ppsum = pp_a.tile([128, 1024], f32, tag='ppsum')
                        for sub in range(2):          # A rows 0:64, B rows 64:128
                            for n5 in range(2):       # N chunks of 512
                                nc.tensor.matmul(
                                    ppsum[sub * 64:(sub + 1) * 64,
                                          n5 * 512:(n5 + 1) * 512],
                                    lhsT=wb["poly_lhsT"][64 * sub:64 * sub + 9, :],
                                    rhs=poly[64 * sub:64 * sub + 9,
                                             n5 * 512:(n5 + 1) * 512],
                                    start=True, stop=True)
                        smear = ep.tile([128, 1024], bf16, tag='smear')
                        nc.scalar.activation(
                            smear[:], ppsum[:], mybir.ActivationFunctionType.Exp,
                            bias=wt["eb"][:])
                        h1p = pp_h1.tile([128, 1024], f32, tag='h1p')
                        for sub in range(2):
                            for n5 in range(2):
                                nc.tensor.matmul(
                                    h1p[sub * 64:(sub + 1) * 64,
                                        n5 * 512:(n5 + 1) * 512],
                                    lhsT=wb["enn1_wT"][sub * 64:sub * 64 + NG, :],
                                    rhs=smear[sub * 64:sub * 64 + NG,
                                              n5 * 512:(n5 + 1) * 512],
                                    start=True, stop=True)
                        h1r = ep.tile([128, 1024], bf16, tag='h1r')
                        nc.scalar.activation(
                            h1r[:], h1p[:], mybir.ActivationFunctionType.Relu,
                            bias=wt["b1s"][:])
                        wtp = pp_a.tile([128, 1024], f32, name='wtp',
                                        tag='ppsum')
                        for sub in range(2):
                            for n5 in range(2):
                                nc.tensor.matmul(
                                    wtp[sub * 64:(sub + 1) * 64,
                                        n5 * 512:(n5 + 1) * 512],
                                    lhsT=wb["enn2_wT"][sub * 64:(sub + 1) * 64, :],
                                    rhs=h1r[sub * 64:(sub + 1) * 64,
                                            n5 * 512:(n5 + 1) * 512],
                                    start=True, stop=True)
                        wts = ep.tile([128, 1024], bf16, tag='wts')
                        nc.scalar.activation(
                            wts[:], wtp[:], mybir.ActivationFunctionType.Identity,
                            bias=wt["b2s"][:])
                        wcp = pp_wc.tile([128, 1024], bf16, tag='wcp')
                        for c in range(8):
                            nc.tensor.transpose(
                                wcp[:, c * 128:(c + 1) * 128],
                                wts[:, c * 128:(c + 1) * 128], ident_bf[:])
                        wcv = wcp[:].rearrange("p (c k) -> p c k", k=128)
                        for sub in range(2):
                            j0 = half * 16 + sub * 8
                            mslice = msg[:, j0:j0 + 8, :]
                            nc.vector.tensor_tensor(
                                out=mslice,
                                in0=gath[:, j0:j0 + 8, :],
                                in1=wcv[:, :, sub * 64:(sub + 1) * 64],
                                op=mybir.AluOpType.mult)
                            cb = cwt[:, j0:j0 + 8].to_broadcast([128, 8, NF])
                            nc.vector.tensor_tensor(
                                out=mslice, in0=mslice, in1=cb,
                                op=mybir.AluOpType.mult)
                            nc.vector.tensor_tensor(
                                out=dmat[:, j0:j0 + 8, :],
                                in0=dstl[:, j0:j0 + 8].to_broadcast([128, 8, 128]),
                                in1=iota_v[:, :8, :],
                                op=mybir.AluOpType.is_equal)
                    # scatter via one-hot matmul, accumulate per cell in PSUM
                    for j in range(32):
                        c = g * 32 + j
                        for (s, k, first, last, _p0, _p1) in chunk_segs[c]:
                            if k == 0:
                                dap = dmat[:, j, :]
                            else:
                                dhi = ep.tile([128, 1, 128], bf16, name='dhi',
                                              tag='dhi')
                                nc.vector.tensor_tensor(
                                    out=dhi[:],
                                    in0=dstl[:, j:j + 1].to_broadcast(
                                        [128, 1, 128]),
                                    in1=iotah_v[:, k - 1:k, :],
                                    op=mybir.AluOpType.is_equal)
                                dap = dhi[:, 0, :]
                            if first:
                                cur_psum[0] = pp_agg.tile([128, NF], f32,
                                                          name='cps', tag='cps')
                            nc.tensor.matmul(
                                cur_psum[0][:],
                                lhsT=dap, rhs=msg[:, j, :],
                                start=bool(first), stop=bool(last))
                            if last:
                                nc.vector.tensor_tensor(
                                    out=agg[:, s, :], in0=cur_psum[0][:],
                                    in1=agg[:, s, :], op=mybir.AluOpType.add)

                y_of_group = cfg["y_of_group"]
                for g in range(min(4, NGROUP)):
                    load_group(g)
                for g in range(NGROUP):
                    if g + 4 < NGROUP:
                        load_group(g + 4)
                    compute_group(g)
                    for t in y_of_group[g]:
                        y_iter(t)
    nc.compile()
    return nc


def run(inputs, n_cores=N_CORES, trace=False, **_ignored):
    from concourse.bass_utils import run_bass_kernel_spmd

    x = np.asarray(inputs["x"], np.float32)
    cfg, per_core = prep_host(x, inputs["edge_index"], inputs["edge_weight"],
                              inputs["edge_attr"], n_cores)
    wts = prep_weights(inputs["lin1_w"], inputs["lin2_w"], inputs["lin2_b"],
                       inputs["enn1_w"], inputs["enn1_b"],
                       inputs["enn2_w"], inputs["enn2_b"])
    nc = build_nc(cfg, n_cores)
    in_maps = [dict(per_core[r], **wts) for r in range(n_cores)]
    res = run_bass_kernel_spmd(nc, in_maps, core_ids=list(range(n_cores)),
                               trace=trace)
    npc = cfg["npc"]
    out = np.concatenate(
        [np.asarray(res.results[r]["out"])[:, :npc].T for r in range(n_cores)],
        axis=0)
    return out, res


def kernel(**inputs):
    out, _ = run(inputs)
    return out


# revision 16
# speedup vs baseline: 1.0392x; 1.0392x over previous
"""Trainium2 Bass kernel for CFConv-style GNN message passing layer.

Full computation (see reference):
  smeared = exp(coeff*(d - offset)^2)            [E, 50]
  W = (relu(smeared @ enn1_w.T + b1) @ enn2_w.T + b2) * C(d)   [E, 64]
  h = x @ lin1_w.T                                [N, 64]
  agg = segment_sum(h[src] * W, dst, N)           [N, 64]
  out = x + relu(agg @ lin2_w.T + lin2_b)         [N, 128]

Sharding: edges partitioned by dst range across 8 cores. Each core computes
h for its node slice, AllGathers h, then processes its own edges.
Edge stream per core is sorted by (src bucket, dst block of 128, src):
  - gather of h rows via gpsimd dma_gather (int16 idx per 32k-row bucket)
  - scatter-add via one-hot matmul: for each 128-edge chunk,
    D[e, m] = (dst_local[e] == m + 128*k) built on DVE via is_equal against
    iota constants; agg_block [128,64] += D.T @ msg accumulated in PSUM
    across the chunks of one (bucket, block) cell. Cells straddling chunk
    boundaries use offset index k>0 for the partial head chunk.
Cell capacities are the exact max across cores so the instruction stream is
SPMD-uniform. The smearing quadratic is evaluated with a K=9 bf16 matmul
using split-precision rows (exact to ~1e-4 in the exponent).
"""

import math

import numpy as np

N_NODES = 100000
N_EDGES = 1600000
DIM = 128
NF = 64
NG = 50
CUTOFF = 5.0
N_CORES = 8

GRP = 4096                      # slots per gather group (= 2 pair-tiles)
CPG = GRP // 128                # chunks per group


def _cfg(n_nodes, n_cores):
    npc = n_nodes // n_cores            # real nodes per core
    assert npc * n_cores == n_nodes
    npad = ((npc + 127) // 128) * 128    # padded nodes per core
    nbg = npad * n_cores                 # global padded node count
    n_buckets = 4
    assert nbg % n_buckets == 0
    bkt = nbg // n_buckets               # gather bucket size (int16 range)
    assert bkt <= 32767
    nsb = npad // 128                    # dst blocks of 128 per core
    return dict(npc=npc, npad=npad, nbg=nbg, n_buckets=n_buckets, bkt=bkt,
                nsb=nsb)


def _bf(a):
    import ml_dtypes
    return a.astype(ml_dtypes.bfloat16)


def _split3(a):
    """Split fp64 array into three bf16 terms summing to ~2^-27 rel."""
    h = _bf(a)
    r = a - h.astype(np.float64)
    m = _bf(r)
    l = _bf(r - m.astype(np.float64))
    return h, m, l


def prep_host(x, edge_index, edge_weight, edge_attr, n_cores=N_CORES):
    """Shard + reorder edges; build per-core input arrays + chunk metadata."""
    cfg = _cfg(x.shape[0], n_cores)
    npc, npad, n_buckets, bkt, nsb = (cfg["npc"], cfg["npad"],
                                      cfg["n_buckets"], cfg["bkt"], cfg["nsb"])

    src = np.asarray(edge_index[0], dtype=np.int64)
    dst = np.asarray(edge_index[1], dtype=np.int64)
    d = np.asarray(edge_attr, dtype=np.float32)

    # global padded src ids laid out to match the two-stage AllGather:
    # stage 0 gathers quarter-slice 0 of every core into rows [0, bkt);
    # stage 1 gathers the remaining 3 quarters into rows [bkt, nbg).
    qn = npad // 4
    bktg = npad * n_cores // 4
    lp = src % npc
    cr = src // npc
    gsrc = np.where(lp < qn, cr * qn + lp,
                    bktg + cr * 3 * qn + (lp - qn))
    core_of = dst // npc

    per_core = []
    counts = np.zeros((n_cores, n_buckets, nsb), dtype=np.int64)
    for r in range(n_cores):
        m = core_of == r
        gs = gsrc[m]
        dl = (dst[m] - r * npc).astype(np.int64)
        dd = d[m]
        bu = gs // bkt
        sb = dl // 128
        order = np.lexsort((gs, sb, bu))
        gs, dl, dd, bu, sb = gs[order], dl[order], dd[order], bu[order], sb[order]
        np.add.at(counts[r], (bu, sb), 1)
        per_core.append((gs, dl, dd, bu, sb))

    caps = counts.max(axis=0)                              # [n_buckets, nsb]
    bucket_len = caps.sum(axis=1)
    bucket_pad = ((bucket_len + GRP - 1) // GRP) * GRP
    bucket_base = np.concatenate([[0], np.cumsum(bucket_pad)[:-1]])
    cell_start = np.zeros((n_buckets, nsb), dtype=np.int64)
    for b in range(n_buckets):
        cell_start[b] = bucket_base[b] + np.concatenate(
            [[0], np.cumsum(caps[b])[:-1]])
    e_pad = int(bucket_pad.sum())
    nchunks = e_pad // 128
    ngroup = e_pad // GRP

    # segment metadata (identical across cores): per chunk, the list of
    # cell segments [sb, k, first, last]
    chunk_segs = [[] for _ in range(nchunks)]
    for b in range(n_buckets):
        for s in range(nsb):
            cap = int(caps[b, s])
            if cap == 0:
                continue
            p0 = int(cell_start[b, s])
            p1 = p0 + cap
            c0, c1 = p0 // 128, (p1 - 1) // 128
            for c in range(c0, c1 + 1):
                chunk_segs[c].append([s, 0, c == c0, c == c1, p0, p1])
    slot_off = np.zeros(e_pad, dtype=np.float32)
    max_k = 0
    for c, segs in enumerate(chunk_segs):
        segs.sort(key=lambda t: max(t[4], c * 128))
        for i, t in enumerate(segs):
            k = 0 if t[4] <= c * 128 else i
            t[1] = k
            max_k = max(max_k, k)
            if k:
                a = max(t[4], c * 128)
                z = min(t[5], (c + 1) * 128)
                slot_off[a:z] = 128.0 * k
    assert max_k <= 2, max_k
    # force cell splits at gather-group boundaries
    for g in range(1, ngroup):
        c = g * CPG
        for t in chunk_segs[c]:
            if not t[2] and t[4] < c * 128:
                t[2] = True
                for t2 in chunk_segs[c - 1]:
                    if t2[4] == t[4]:
                        t2[3] = True

    cfg["e_pad"] = e_pad
    cfg["ngroup"] = ngroup
    cfg["chunk_segs"] = chunk_segs
    cfg["group_bucket"] = np.searchsorted(
        np.cumsum(bucket_pad), np.arange(ngroup) * GRP, side="right")
    # group after which each y-iteration (pair of dst blocks) is ready
    last_group = np.zeros(nsb, dtype=np.int64)
    for s in range(nsb):
        for b in range(n_buckets):
            if caps[b, s] > 0:
                last_group[s] = max(
                    last_group[s],
                    (int(cell_start[b, s]) + int(caps[b, s]) - 1) // GRP)
    y_of_group = [[] for _ in range(ngroup)]
    for t in range(nsb // 2):
        y_of_group[max(last_group[2 * t], last_group[2 * t + 1])].append(t)
    cfg["y_of_group"] = y_of_group

    nt = e_pad // 2048  # pair-tiles
    cell_flat = cell_start.reshape(-1)
    coeff = -0.5 / (CUTOFF / (NG - 1)) ** 2
    ins = []
    for r in range(n_cores):
        gs, dl, dd, bu, sb = per_core[r]
        key = bu * nsb + sb
        n = len(key)
        if n:
            newgrp = np.r_[True, key[1:] != key[:-1]]
            starts = np.nonzero(newgrp)[0]
            lens = np.diff(np.r_[starts, n])
            cc = np.arange(n) - np.repeat(starts, lens)
        else:
            cc = np.zeros(0, dtype=np.int64)
        pos = cell_flat[key] + cc

        srcl = np.zeros(e_pad, dtype=np.int16)
        dloc = np.full(e_pad, 99999.0, dtype=np.float32)
        dpad = np.full(e_pad, CUTOFF, dtype=np.float32)
        srcl[pos] = (gs - bu * bkt).astype(np.int16)
        dloc[pos] = (dl % 128).astype(np.float32)
        dloc += slot_off
        dpad[pos] = dd

        d64 = dpad.astype(np.float64)
        cw = (0.5 * (np.cos(dpad * (math.pi / CUTOFF)) + 1.0)).astype(np.float32)
        uh, um, ul = _split3(coeff * d64 * d64)
        vh, vm, vl = _split3(-2.0 * coeff * d64)
        # poly rows per half: [uh um ul vh vm vl vh vm vh]  (9 bf16 rows)
        rows = [uh, um, ul, vh, vm, vl, vh, vm, vh]
        poly = np.empty((nt, 18, 1024), dtype=uh.dtype)
        for i, rr_ in enumerate(rows):
            rv = rr_.reshape(nt, 2, 1024)
            poly[:, i, :] = rv[:, 0, :]
            poly[:, 9 + i, :] = rv[:, 1, :]
        xt = np.zeros((DIM, npad), dtype=np.float32)
        xt[:, :npc] = np.asarray(x[r * npc:(r + 1) * npc, :], np.float32).T
        ins.append(dict(
            xT=np.ascontiguousarray(xt),
            poly=np.ascontiguousarray(poly),
            cw=cw,
            srcidx=np.ascontiguousarray(np.tile(srcl.reshape(-1, 16).T, (8, 1))),
            dstloc=np.ascontiguousarray(dloc.reshape(-1, 128).T),
        ))
    return cfg, ins


def _stack2(w, rows):
    out = np.zeros((rows, w.shape[1]), dtype=np.float32)
    out[:w.shape[0]] = w
    out[64:64 + w.shape[0]] = w
    return out


def prep_weights(lin1_w, lin2_w, lin2_b, enn1_w, enn1_b, enn2_w, enn2_b):
    """Constant (per-core-identical) weight arrays."""
    offset = np.linspace(0.0, CUTOFF, NG).astype(np.float64)
    coeff = -0.5 / (offset[1] - offset[0]) ** 2
    oh, om, ol = _split3(offset)
    one = np.ones(NG, dtype=np.float64)
    # lhsT rows pair with poly rows: [1 1 1 oh oh oh om om ol]
    lrows = [one, one, one, oh, oh, oh, om, om, ol]
    poly_lhsT = np.zeros((73, 64), dtype=np.float32)
    for b0 in (0, 64):
        for i, lr in enumerate(lrows):
            poly_lhsT[b0 + i, :NG] = np.asarray(lr, np.float32)
    eb = np.full((128, 1), -88.0, dtype=np.float32)
    eb[:NG, 0] = (coeff * offset * offset).astype(np.float32)
    eb[64:64 + NG, 0] = eb[:NG, 0]
    b1s = np.zeros((128, 1), dtype=np.float32)
    b1s[:NF, 0] = enn1_b
    b1s[64:64 + NF, 0] = enn1_b
    b2s = np.zeros((128, 1), dtype=np.float32)
    b2s[:NF, 0] = enn2_b
    b2s[64:64 + NF, 0] = enn2_b
    iota8 = np.tile(np.arange(128, dtype=np.float32)[None, :], (128, 8))
    iotah = np.tile(np.arange(128, 384, dtype=np.float32)[None, :], (128, 1))
    return dict(
        lin1_wT=np.ascontiguousarray(lin1_w.T.astype(np.float32)),    # [128, 64]
        lin2_wT=np.ascontiguousarray(lin2_w.T.astype(np.float32)),    # [64, 128]
        enn1_wT=_stack2(enn1_w.T.astype(np.float32), 114),            # [114, 64]
        enn2_wT=_stack2(enn2_w.T.astype(np.float32), 128),            # [128, 64]
        poly_lhsT=poly_lhsT,
        eb=eb, b1s=b1s, b2s=b2s,
        ident=np.eye(128, dtype=np.float32),
        iota8=np.ascontiguousarray(iota8),                            # [128, 1024]
        iotah=np.ascontiguousarray(iotah),                            # [128, 256]
        l2b=np.ascontiguousarray(lin2_b.astype(np.float32).reshape(128, 1)),
    )


def build_nc(cfg, n_cores=N_CORES, **_ignored):
    import concourse.bass as bass
    import concourse.bacc as bacc
    import concourse.mybir as mybir
    import concourse.tile as tile
    from concourse import library_config

    f32 = mybir.dt.float32
    bf16 = mybir.dt.bfloat16
    i16 = mybir.dt.int16
    npad, nbg, bkt, e_pad = cfg["npad"], cfg["nbg"], cfg["bkt"], cfg["e_pad"]
    nsb = cfg["nsb"]
    NT = e_pad // 2048            # pair-tiles
    NGROUP = cfg["ngroup"]
    chunk_segs = cfg["chunk_segs"]
    group_bucket = cfg["group_bucket"]

    nc = bacc.Bacc(None, num_devices=n_cores)

    # I/O
    xT_d = nc.dram_tensor("xT", [DIM, npad], f32, kind="ExternalInput")
    poly_d = nc.dram_tensor("poly", [NT, 18, 1024], bf16, kind="ExternalInput")
    cw_d = nc.dram_tensor("cw", [e_pad], f32, kind="ExternalInput")
    sidx_d = nc.dram_tensor("srcidx", [128, e_pad // 16], i16, kind="ExternalInput")
    dstloc_d = nc.dram_tensor("dstloc", [128, e_pad // 128], f32,
                              kind="ExternalInput")
    w_d = {}
    for name, shape in [("lin1_wT", [DIM, NF]), ("lin2_wT", [NF, DIM]),
                        ("enn1_wT", [114, NF]), ("enn2_wT", [128, NF]),
                        ("poly_lhsT", [73, 64]), ("eb", [128, 1]),
                        ("b1s", [128, 1]), ("b2s", [128, 1]), ("l2b", [128, 1]),
                        ("ident", [128, 128]), ("iota8", [128, 1024]),
                        ("iotah", [128, 256])]:
        w_d[name] = nc.dram_tensor(name, shape, f32, kind="ExternalInput")
    out_d = nc.dram_tensor("out", [DIM, npad], f32, kind="ExternalOutput")

    h_self = nc.dram_tensor("h_self", [npad, NF], f32)
    h_full = nc.dram_tensor("h_full", [nbg, NF], f32)

    with tile.TileContext(nc) as tc:
        with tc.tile_pool(name="const", bufs=1) as cp:
            wt = {}
            for name in w_d:
                t = cp.tile(list(w_d[name].shape), f32, tag='w_' + name)
                nc.sync.dma_start(out=t[:], in_=w_d[name][:, :])
                wt[name] = t
            ident = wt["ident"]
            # bf16 copies for the bf16 matmul path
            wb = {}
            for name in ("enn1_wT", "enn2_wT", "ident", "poly_lhsT"):
                t = cp.tile(list(w_d[name].shape), bf16, tag='wb_' + name)
                nc.scalar.activation(t[:], wt[name][:],
                                     mybir.ActivationFunctionType.Copy)
                wb[name] = t
            ident_bf = wb["ident"]
            nc.gpsimd.load_library(library_config.mlp)
            agg = cp.tile([128, nsb, NF], f32, tag='agg')
            nc.vector.memset(agg[:], 0.0)

            # ---------------- h phase ----------------
            with (tc.tile_pool(name="hp", bufs=3) as hp,
                  tc.tile_pool(name="hpp", bufs=2, space="PSUM") as hpp):
                for s in range(npad // 256):
                    xt = hp.tile([128, 256], f32, tag='xt')
                    nc.sync.dma_start(out=xt[:], in_=xT_d[:, s * 256:(s + 1) * 256])
                    hps = hpp.tile([128, 128], f32, tag='hps')
                    for c in range(2):
                        nc.tensor.matmul(
                            hps[:, c * 64:(c + 1) * 64],
                            lhsT=xt[:, c * 128:(c + 1) * 128],
                            rhs=wt["lin1_wT"][:],
                            start=True, stop=True)
                    hsb = hp.tile([128, 128], f32, tag='hsb')
                    nc.scalar.activation(hsb[:], hps[:],
                                         mybir.ActivationFunctionType.Copy)
                    dst_ap = h_self[s * 256:(s + 1) * 256, :].rearrange(
                        "(c p) f -> p c f", p=128)
                    nc.sync.dma_start(
                        out=dst_ap,
                        in_=hsb[:].rearrange("p (c f) -> p c f", f=NF))

            qn = npad // 4
            nc.gpsimd.collective_compute(
                "AllGather", mybir.AluOpType.bypass,
                replica_groups=[list(range(n_cores))],
                ins=[h_self[0:qn, :].opt()],
                outs=[h_full[0:bkt, :].opt()])
            nc.gpsimd.collective_compute(
                "AllGather", mybir.AluOpType.bypass,
                replica_groups=[list(range(n_cores))],
                ins=[h_self[qn:npad, :].opt()],
                outs=[h_full[bkt:nbg, :].opt()])

            # ---------------- edge phase ----------------
            with (tc.tile_pool(name="gp", bufs=3) as gp,
                  tc.tile_pool(name="ep", bufs=2) as ep,
                  tc.tile_pool(name="yp", bufs=2) as yp,
                  tc.tile_pool(name="pp_a", bufs=1, space="PSUM") as pp_a,
                  tc.tile_pool(name="pp_h1", bufs=1, space="PSUM") as pp_h1,
                  tc.tile_pool(name="pp_wc", bufs=1, space="PSUM") as pp_wc,
                  tc.tile_pool(name="pp_agg", bufs=1, space="PSUM") as pp_agg):
                pend = {}
                cur_psum = [None]

                def y_iter(s):
                    yt = pp_agg.tile([128, 512], f32, name='yt', tag='ytile')
                    atp = yt[0:64, 256:512]
                    for j in range(2):
                        blk = 2 * s + j
                        nc.tensor.transpose(
                            atp[:, j * 128:(j + 1) * 128], agg[:, blk, :],
                            ident[:])
                    ats = yp.tile([64, 256], f32, name='ats', tag='ats')
                    nc.scalar.activation(ats[:], atp[:],
                                         mybir.ActivationFunctionType.Copy)
                    ytp = yt[:, 0:256]
                    nc.tensor.matmul(ytp, lhsT=wt["lin2_wT"][:],
                                     rhs=ats[:], start=True, stop=True)
                    yr = yp.tile([128, 256], f32, name='yr', tag='yr')
                    nc.scalar.activation(yr[:], ytp,
                                         mybir.ActivationFunctionType.Relu,
                                         bias=wt["l2b"][:])
                    xt2 = yp.tile([128, 256], f32, name='xt2', tag='xt2')
                    nc.sync.dma_start(out=xt2[:],
                                      in_=xT_d[:, s * 256:(s + 1) * 256])
                    ot = yp.tile([128, 256], f32, name='ot', tag='ot')
                    nc.vector.tensor_tensor(out=ot[:], in0=yr[:], in1=xt2[:],
                                            op=mybir.AluOpType.add)
                    nc.sync.dma_start(out=out_d[:, s * 256:(s + 1) * 256],
                                      in_=ot[:])

                def load_group(g):
                    b = int(group_bucket[g])
                    sidx = gp.tile([128, 256], i16, tag='sidx')
                    nc.sync.dma_start(
                        out=sidx[:],
                        in_=sidx_d[:, g * 256:(g + 1) * 256])
                    dstl = gp.tile([128, 32], f32, tag='dstl')
                    nc.sync.dma_start(
                        out=dstl[:],
                        in_=dstloc_d[:, g * 32:(g + 1) * 32])
                    cwt = gp.tile([128, 32], f32, tag='cwt')
                    nc.sync.dma_start(
                        out=cwt[:],
                        in_=cw_d[g * 4096:(g + 1) * 4096].rearrange(
                            "(c p) -> p c", p=128))
                    gath = gp.tile([128, 32, NF], f32, tag='gath')
                    nc.gpsimd.dma_gather(
                        gath[:], h_full[b * bkt:(b + 1) * bkt, :], sidx[:],
                        num_idxs=4096, num_idxs_reg=4096, elem_size=NF,
                        single_packet=False)
                    pend[g] = (gath, dstl, cwt)

                def compute_group(g):
                    gath, dstl, cwt = pend.pop(g)
                    msg = ep.tile([128, 32, NF], bf16, tag='msg')
                    dmat = ep.tile([128, 32, 128], bf16, tag='dmat')
                    iota_v = wt["iota8"][:].rearrange("p (c k) -> p c k", k=128)
                    iotah_v = wt["iotah"][:].rearrange("p (c k) -> p c k", k=128)
                    for half in range(2):
                        t = 2 * g + half
                        poly = ep.tile([73, 1024], bf16, tag='poly')
                        nc.sync.dma_start(out=poly[0:9, :], in_=poly_d[t, 0:9, :])
                        nc.sync.dma_start(out=poly[64:73, :],
                                          in_=poly_d[t, 9:18, :])
                        ppsum = pp_a.tile([128, 1024], f32, tag='ppsum')
                        for sub in range(2):          # A rows 0:64, B rows 64:128
                            for n5 in range(2):       # N chunks of 512
                                nc.tensor.matmul(
                                    ppsum[sub * 64:(sub + 1) * 64,
                                          n5 * 512:(n5 + 1) * 512],
                                    lhsT=wb["poly_lhsT"][64 * sub:64 * sub + 9, :],
                                    rhs=poly[64 * sub:64 * sub + 9,
                                             n5 * 512:(n5 + 1) * 512],
                                    start=True, stop=True)
                        smear = ep.tile([128, 1024], bf16, tag='smear')
                        nc.scalar.activation(
                            smear[:], ppsum[:], mybir.ActivationFunctionType.Exp,
                            bias=wt["eb"][:])
                        h1p = pp_h1.tile([128, 1024], f32, tag='h1p')
                        for sub in range(2):
                            for n5 in range(2):
                                nc.tensor.matmul(
                                    h1p[sub * 64:(sub + 1) * 64,
                                        n5 * 512:(n5 + 1) * 512],
                                    lhsT=wb["enn1_wT"][sub * 64:sub * 64 + NG, :],
                                    rhs=smear[sub * 64:sub * 64 + NG,
                                              n5 * 512:(n5 + 1) * 512],
                                    start=True, stop=True)
                        h1r = ep.tile([128, 1024], bf16, tag='h1r')
                        nc.scalar.activation(
                            h1r[:], h1p[:], mybir.ActivationFunctionType.Relu,
                            bias=wt["b1s"][:])
                        wtp = pp_a.tile([128, 1024], f32, name='wtp',
                                        tag='ppsum')
                        for sub in range(2):
                            for n5 in range(2):
                                nc.tensor.matmul(
                                    wtp[sub * 64:(sub + 1) * 64,
                                        n5 * 512:(n5 + 1) * 512],
                                    lhsT=wb["enn2_wT"][sub * 64:(sub + 1) * 64, :],
                                    rhs=h1r[sub * 64:(sub + 1) * 64,
                                            n5 * 512:(n5 + 1) * 512],
                                    start=True, stop=True)
                        wts = ep.tile([128, 1024], bf16, tag='wts')
                        nc.scalar.activation(
                            wts[:], wtp[:], mybir.ActivationFunctionType.Identity,
                            bias=wt["b2s"][:])
                        wcp = pp_wc.tile([128, 1024], bf16, tag='wcp')
                        for c in range(8):
                            nc.tensor.transpose(
                                wcp[:, c * 128:(c + 1) * 128],
                                wts[:, c * 128:(c + 1) * 128], ident_bf[:])
                        wcv = wcp[:].rearrange("p (c k) -> p c k", k=128)
                        for sub in range(2):
                            j0 = half * 16 + sub * 8
                            mslice = msg[:, j0:j0 + 8, :]
                            nc.vector.tensor_tensor(
                                out=mslice,
                                in0=gath[:, j0:j0 + 8, :],
                                in1=wcv[:, :, sub * 64:(sub + 1) * 64],
                                op=mybir.AluOpType.mult)
                            cb = cwt[:, j0:j0 + 8].to_broadcast([128, 8, NF])
                            nc.vector.tensor_tensor(
                                out=mslice, in0=mslice, in1=cb,
                                op=mybir.AluOpType.mult)
                            nc.vector.tensor_tensor(
                                out=dmat[:, j0:j0 + 8, :],
                                in0=dstl[:, j0:j0 + 8].to_broadcast([128, 8, 128]),
                                in1=iota_v[:, :8, :],
                                op=mybir.AluOpType.is_equal)
                    # scatter via one-hot matmul, accumulate per cell in PSUM
                    for j in range(32):
                        c = g * 32 + j
                        for (s, k, first, last, _p0, _p1) in chunk_segs[c]:
                            if k == 0:
                                dap = dmat[:, j, :]
                            else:
                                dhi = ep.tile([128, 1, 128], bf16, name='dhi',
                                              tag='dhi')
                                nc.vector.tensor_tensor(
                                    out=dhi[:],
                                    in0=dstl[:, j:j + 1].to_broadcast(
                                        [128, 1, 128]),
                                    in1=iotah_v[:, k - 1:k, :],
                                    op=mybir.AluOpType.is_equal)
                                dap = dhi[:, 0, :]
                            if first:
                                cur_psum[0] = pp_agg.tile([128, NF], f32,
                                                          name='cps', tag='cps')
                            nc.tensor.matmul(
                                cur_psum[0][:],
                                lhsT=dap, rhs=msg[:, j, :],
                                start=bool(first), stop=bool(last))
                            if last:
                                nc.vector.tensor_tensor(
                                    out=agg[:, s, :], in0=cur_psum[0][:],
                                    in1=agg[:, s, :], op=mybir.AluOpType.add)

                y_of_group = cfg["y_of_group"]
                for g in range(min(3, NGROUP)):
                    load_group(g)
                for g in range(NGROUP):
                    if g + 3 < NGROUP:
                        load_group(g + 3)
                    compute_group(g)
                    for t in y_of_group[g]:
                        y_iter(t)
    nc.compile()
    return nc


def run(inputs, n_cores=N_CORES, trace=False, **_ignored):
    from concourse.bass_utils import run_bass_kernel_spmd

    x = np.asarray(inputs["x"], np.float32)
    cfg, per_core = prep_host(x, inputs["edge_index"], inputs["edge_weight"],
                              inputs["edge_attr"], n_cores)
    wts = prep_weights(inputs["lin1_w"], inputs["lin2_w"], inputs["lin2_b"],
                       inputs["enn1_w"], inputs["enn1_b"],
                       inputs["enn2_w"], inputs["enn2_b"])
    nc = build_nc(cfg, n_cores)
    in_maps = [dict(per_core[r], **wts) for r in range(n_cores)]
    res = run_bass_kernel_spmd(nc, in_maps, core_ids=list(range(n_cores)),
                               trace=trace)
    npc = cfg["npc"]
    out = np.concatenate(
        [np.asarray(res.results[r]["out"])[:, :npc].T for r in range(n_cores)],
        axis=0)
    return out, res


def kernel(**inputs):
    out, _ = run(inputs)
    return out
